# revision 32
# baseline (speedup 1.0000x reference)
"""Trainium2 Bass kernel for nn_DomainAttention (moe_routing).

Math (see reference):
    con[n,b]  = cat[n] . x[b]                       # [N, B]
    con      /= max(||con[:,b]||_4, 1e-12)          # 4-norm over N, per column
    p         = softmax(con, axis=N)
    w[s,b]    = sum_{n in chunk s} y[n] * p[n,b]
    theta[s,b]= exp(x[b] . phi[s])
    out[b]    = sigmoid(sum_s w[s,b]*theta[s,b] + bias)

Device strategy (8 NeuronCores, data-parallel over B, 512 columns/core):
  - con computed as [b_part=128, n_free] tiles: lhsT = x^T (stationary), rhs =
    cat^T (moving), fp8e4m3 inputs with DoubleRow perf mode (two 128-deep
    contraction sub-rows per matmul -> half the matmul count), fp32 PSUM
    accumulation.  cat^T and x^T stay resident in SBUF; quantization noise
    washes out in the 2048-element softmax sums (measured ~6e-5 rel err).
  - |con|/norm4 <= 1 always (norm4 >= max|con|), so softmax needs no
    max-subtraction: e = exp(con * inv4), p = e / Z.
  - s4 = sum con^4 partials via a runtime-registered custom DVE op
    (out=(x^2)^2, accum=sum) - one Vector pass per chunk.
  - inv4 = s4^(-1/4) via exponent-shift seed + 2 Newton steps on DVE (no
    Ln/Sqrt -> a single ACT table set for the whole kernel; the ACT
    accumulator path is avoided everywhere: each read costs a ~1.5us flush).
  - e ships to DRAM; the host does the w_s/F_s/Z sums, theta, bias and
    sigmoid in f64 (the n-permutation puts y==1 first per source chunk so
    w_s is a prefix sum; DMA queues are otherwise idle).
  - PE clock warm-up: junk matmuls during the DMA fill hold the HAM gate at
    2.4 GHz so the real stream never runs throttled.
  - Engine split: PSUM drains fp32->bf16 con copies on ACT (phases A/B) and
    DVE (late phases, where ACT runs the exps); pass2 work is spliced into
    the NEXT phase's emission slots because engine queues are strict FIFO.
"""
import os

os.environ.setdefault("JAX_PLATFORMS", "axon,cpu")

from contextlib import ExitStack

import ml_dtypes
import numpy as np

import operator

import concourse.bass as bass  # noqa: F401
import concourse.tile as tile
from concourse import bacc, bass_utils, mybir
from concourse import dve_ops as _dve_ops
from concourse.dve_spec import C0 as _C0
from concourse.dve_spec import Spec as _Spec
from concourse.dve_spec import Src0 as _Src0
from concourse.dve_spec import lower as _dve_lower
from concourse.dve_spec import sq as _sq
from concourse.dve_table_gen import dve_ver_for as _dve_ver_for
from concourse.dve_uop import DveOpSpec as _DveOpSpec


def _get_quad_op():
    """Register (once, at runtime) a custom DVE op computing
    out = (in0^2)^2 with accum_out = c0 + sum(out) — the s4 partial sums in a
    single Vector pass instead of square + square-accumulate."""
    name = "QUAD_REDUCE_ANT_K"
    for o in _dve_ops.OPS:
        if o.name == name:
            return o
    spec = _Spec(
        body=_sq(_sq(_Src0)),
        accum=operator.add,
        accum_init=_C0,
        reference=_dve_ops._ref_body_sum(
            lambda in0, in1, c0, c1, c2: np.square(np.square(in0.astype(np.float32)))
        ),
    )
    row = _dve_ops._CUSTOM_DVE_ROW_BASE + len(_dve_ops.OPS)
    _dve_ops._SUB_OPCODE_FOR_NAME[name] = row
    ver = _dve_ver_for("TRN2")
    sha = _DveOpSpec(
        name=name, opcode=row, uops=_dve_lower(spec, ver=ver), rd1_en=False
    ).sha(ver)
    op = _dve_ops.DveOp(
        name, spec, subdim=False, uops_sha={ver: sha}, perf_en={ver: True}
    )
    _dve_ops.OPS.append(op)
    _dve_ops.CUSTOM_DVE_SPECS[name] = spec
    return op

B, D, N, S = 4096, 768, 8192, 4
NCORES = 8
P = 128
BL = B // NCORES          # 512 batch columns per core
NBT = BL // P             # 4 b-tiles per core
NDC = D // P              # 6 contraction chunks
CHUNK = N // S            # 2048 (source chunk along n)
G8 = 2048                 # psum drain chunk along n
NG8 = N // G8             # 4

# Magic constant for the y0 ~= x^(-1/4) exponent trick (fast-inverse-sqrt
# style): bits(y0) = K - (bits(x) >> 2).
_QROOT_K = int(round(1.25 * (2 ** 23) * (127 - 0.0450466)))

_F32 = mybir.dt.float32
_BF16 = mybir.dt.bfloat16
_I32 = mybir.dt.int32
_FP8 = mybir.dt.float8e4

_QUAD = _get_quad_op()

_cache: dict = {}


def _emit(ctx, tc, xT, catT, e_out, ks):
    nc = tc.nc
    AX = mybir.AxisListType.X
    OP = mybir.AluOpType
    AF = mybir.ActivationFunctionType

    cat_pool = ctx.enter_context(tc.tile_pool(name="cat", bufs=4))
    x_pool = ctx.enter_context(tc.tile_pool(name="xp", bufs=1))
    con_pool = ctx.enter_context(tc.tile_pool(name="conp", bufs=NBT))
    dmp_pool = ctx.enter_context(tc.tile_pool(name="dmp", bufs=3))
    e_pool = ctx.enter_context(tc.tile_pool(name="ep", bufs=2))
    st_pool = ctx.enter_context(tc.tile_pool(name="st", bufs=1))
    ps_pool = ctx.enter_context(tc.tile_pool(name="ps", bufs=2, space="PSUM"))

    # x^T resident: xT_sb[p, dc*BL + b] = xT[dc*128+p, b]
    xT_sb = x_pool.tile([P, NDC * BL], _FP8, name="xT_sb")
    for dc in range(NDC):
        nc.sync.dma_start(xT_sb[:, dc * BL:(dc + 1) * BL], xT[dc * P:(dc + 1) * P, :])

    # cat^T resident: [128, 2048] per (g4, dc).  The first g4 group is pulled
    # in 512-wide quarters so the first accumulation groups can start sooner.
    cat_sb = {}
    for g4 in range(4):
        cat_sb[g4] = cat_pool.tile([P, NDC * 2048], _FP8, name=f"cat_{g4}", tag="cat")
    for dc in range(NDC):
        for q in range(2):
            nc.sync.dma_start(
                cat_sb[0][:, dc * 2048 + q * 1024:dc * 2048 + (q + 1) * 1024],
                catT[dc * P:(dc + 1) * P, q * 1024:(q + 1) * 1024],
            )
    for g4 in range(1, 4):
        for dc in range(NDC):
            nc.sync.dma_start(
                cat_sb[g4][:, dc * 2048:(dc + 1) * 2048],
                catT[dc * P:(dc + 1) * P, g4 * 2048:(g4 + 1) * 2048],
            )

    # PE clock warm-up: the HAM gate holds a cold PE at 1.2 GHz until ~3.4us
    # of sustained activity, and re-throttles after ~3.4us idle.  Run junk
    # matmuls against the early-arriving xT tile through the cat DMA fill so
    # the real stream starts (and stays) at 2.4 GHz.
    warm_ps = ps_pool.tile([P, 512], _F32, name="warm_ps", tag="ps")
    for _ in range(18):
        nc.tensor.matmul(warm_ps, xT_sb[:, 0:P], xT_sb[:, 0:512], start=True, stop=True)
    warm_sink = st_pool.tile([P, 1], _F32, name="warm_sink")
    nc.vector.tensor_copy(warm_sink, warm_ps[:, 0:1])

    con_sb = [con_pool.tile([P, N], _BF16, name=f"con{bt}", tag="con") for bt in range(NBT)]
    s4p = [st_pool.tile([P, NG8], _F32, name=f"s4p{bt}") for bt in range(NBT)]

    def mm_chunk(bt, g8):
        """12 DoubleRow matmuls accumulating con[bt, g8*2048:(g8+1)*2048] in
        PSUM (3 k-chunks of 256 x 4 n-slices of 512), then drain: fp32->bf16
        con copy (ACT or DVE) + fused quad-reduce s4 partial (DVE)."""
        ps = ps_pool.tile([P, G8], _F32, name="ps")
        xT_r = xT_sb.rearrange("p (c b) -> p c b", c=NDC)
        cat_r = cat_sb[g8].rearrange("p (c n) -> p c n", c=NDC)
        for dc in range(NDC // 2):
            lhsT = xT_r[:, 2 * dc:2 * dc + 2, bt * P:(bt + 1) * P]
            for h in range(4):
                nc.tensor.matmul(
                    ps[:, h * 512:(h + 1) * 512],
                    lhsT,
                    cat_r[:, 2 * dc:2 * dc + 2, h * 512:(h + 1) * 512],
                    start=(dc == 0),
                    stop=(dc == NDC // 2 - 1),
                    perf_mode=mybir.MatmulPerfMode.DoubleRow,
                )
        cs = con_sb[bt][:, g8 * G8:(g8 + 1) * G8]
        # Phase B/C drains go to DVE: ACT is busy with the softmax exps there,
        # while DVE only runs the fused quad.
        if bt >= 2 and g8 >= 2:
            nc.vector.tensor_copy(cs, ps)
        else:
            nc.scalar.activation(cs, ps, AF.Copy)
        # s4 partial in ONE fused DVE pass (bf16 con input -> 2x perf mode).
        dmp = dmp_pool.tile([P, G8], _BF16, name="dmp")
        nc.vector._custom_dve(
            _QUAD, out=dmp, in0=cs, s0=0.0, s1=0.0, imm2=0.0,
            accum_out=s4p[bt][:, g8:g8 + 1],
        )

    def newton_closure(bt):
        def emit():
            # s4 = sum of chunk partials; inv4 = s4^(-1/4) via bit trick + Newton.
            eng = nc.vector
            s4 = st_pool.tile([P, 1], _F32, name=f"s4_{bt}")
            nc.vector.tensor_reduce(s4, s4p[bt], axis=AX, op=OP.add)
            eng.tensor_scalar_max(s4, s4, 1e-30)
            y = st_pool.tile([P, 1], _F32, name=f"y_{bt}")
            eng.tensor_scalar(y.bitcast(_I32), s4.bitcast(_I32), 2, None,
                              op0=OP.arith_shift_right)
            eng.tensor_scalar(y.bitcast(_I32), y.bitcast(_I32), -1, _QROOT_K,
                              op0=OP.mult, op1=OP.add)
            y2 = st_pool.tile([P, 1], _F32, name=f"y2_{bt}")
            u = st_pool.tile([P, 1], _F32, name=f"u_{bt}")
            for _ in range(2):
                eng.tensor_tensor(y2, y, y, op=OP.mult)
                eng.tensor_tensor(y2, y2, y2, op=OP.mult)      # y^4
                eng.tensor_tensor(u, y2, s4, op=OP.mult)       # a*y^4
                eng.tensor_scalar(u, u, -0.25, 1.25, op0=OP.mult, op1=OP.add)
                eng.tensor_tensor(y, y, u, op=OP.mult)
            inv4[bt] = y
        return emit

    def exp_closure(bt):
        def emit():
            # One exp over the whole b-tile row (plain ACTIVATE -- the
            # accumulator path costs a ~1.5us pipeline flush per read); e
            # ships to DRAM in quarter DMAs and the host does the w/F sums
            # in f64 (DMA queues are otherwise idle).
            y = inv4[bt]
            e = e_pool.tile([P, N], _BF16, name="e", tag="e")
            nc.scalar.activation(e, con_sb[bt], AF.Exp, scale=y)
            for q in range(4):
                nc.sync.dma_start(
                    e_out[:, bt * N + q * CHUNK:bt * N + (q + 1) * CHUNK],
                    e[:, q * CHUNK:(q + 1) * CHUNK],
                )
        return emit

    inv4 = {}
    # Emission order == engine FIFO order (Tile schedules by program order).
    # pass2 work of a phase is spliced into the NEXT phase's chunk slots so
    # exp ops never sit in ACT's FIFO ahead of psum-drain copies whose psum
    # slots the TensorEngine is waiting to reuse.
    pending = []
    for phase in ((0, 1), (2,), (3,)):
        for g8 in range(NG8):
            for bt in phase:
                mm_chunk(bt, g8)
            if pending:
                pending.pop(0)()
        for bt in phase:
            # inv4 chain runs NOW (ahead of the next phase's DVE work in the
            # FIFO) so the big exp can overlap the next phase's matmuls.
            newton_closure(bt)()
            pending.append(exp_closure(bt))
    for fn in pending:
        fn()



def build_program(ks=None):
    key = "prog"
    if key in _cache:
        return _cache[key]
    nc = bacc.Bacc("TRN2", target_bir_lowering=False, debug=False, num_devices=NCORES)
    xT = nc.dram_tensor("xTl", [D, BL], _FP8, kind="ExternalInput").ap()
    catT = nc.dram_tensor("catTp", [D, N], _FP8, kind="ExternalInput").ap()
    e_out = nc.dram_tensor("e_out", [P, NBT * N], _BF16, kind="ExternalOutput").ap()
    with tile.TileContext(nc) as tc, ExitStack() as ctx:
        _emit(ctx, tc, xT, catT, e_out, ks)
    nc.compile()
    _cache[key] = nc
    return nc


def host_prep(batch_x, cat, y):
    """Permute n within each source chunk (y==1 first), build bf16 transposed
    inputs. Returns (catT_bf16 [D,N], xT_bf16 [D,B], ks)."""
    y = np.asarray(y)
    perm = np.empty(N, dtype=np.int64)
    ks = []
    for s in range(S):
        ys = y[s * CHUNK:(s + 1) * CHUNK]
        order = np.argsort(ys == 0, kind="stable")  # nonzero first
        perm[s * CHUNK:(s + 1) * CHUNK] = s * CHUNK + order
        ks.append(int((ys != 0).sum()))
    catp = np.asarray(cat)[perm]
    catT = np.ascontiguousarray(catp.T).astype(ml_dtypes.float8_e4m3)
    xT = np.ascontiguousarray(np.asarray(batch_x).T).astype(ml_dtypes.float8_e4m3)
    return catT, xT, ks


def host_epilogue(results, batch_x, phi, bias, ks):
    """results: list over cores of {'e_out': [128, NBT*N] bf16}. Host computes
    w_s (prefix sums), F_s, Z, theta, bias, sigmoid in f64."""
    theta = np.exp(np.asarray(batch_x, np.float64) @ np.asarray(phi, np.float64).T)
    out = np.empty(B, np.float64)
    for c in range(NCORES):
        e = np.asarray(results[c]["e_out"]).astype(np.float64)
        e = e.reshape(P, NBT, S, CHUNK)
        f = e.sum(axis=3)                       # [P, NBT, S]
        z = f.sum(axis=2)                       # [P, NBT]
        for bt in range(NBT):
            bidx = c * BL + bt * P + np.arange(P)
            w = np.stack(
                [e[:, bt, s, :ks[s]].sum(axis=1) for s in range(S)], axis=1
            )                                   # [P, S]
            out[bidx] = ((w / z[:, bt:bt + 1]) * theta[bidx, :]).sum(axis=1)
    out = out + float(np.asarray(bias).reshape(-1)[0])
    return (1.0 / (1.0 + np.exp(-out))).astype(np.float32)


def make_in_maps(catT, xT):
    return [
        {
            "catTp": catT,
            "xTl": np.ascontiguousarray(xT[:, c * BL:(c + 1) * BL]),
        }
        for c in range(NCORES)
    ]


def kernel(batch_x, cat, y, phi, bias):
    catT, xT, ks = host_prep(batch_x, cat, y)
    nc = build_program(ks)
    res = bass_utils.run_bass_kernel_spmd(nc, make_in_maps(catT, xT), core_ids=list(range(NCORES)))
    return host_epilogue(res.results, batch_x, phi, bias, ks)


# revision 33
# speedup vs baseline: 1.0086x; 1.0086x over previous
"""Trainium2 Bass kernel for nn_DomainAttention (moe_routing).

Math (see reference):
    con[n,b]  = cat[n] . x[b]                       # [N, B]
    con      /= max(||con[:,b]||_4, 1e-12)          # 4-norm over N, per column
    p         = softmax(con, axis=N)
    w[s,b]    = sum_{n in chunk s} y[n] * p[n,b]
    theta[s,b]= exp(x[b] . phi[s])
    out[b]    = sigmoid(sum_s w[s,b]*theta[s,b] + bias)

Device strategy (8 NeuronCores, data-parallel over B, 512 columns/core):
  - con computed as [b_part=128, n_free] tiles: lhsT = x^T (stationary), rhs =
    cat^T (moving), fp8e4m3 inputs with DoubleRow perf mode (two 128-deep
    contraction sub-rows per matmul -> half the matmul count), fp32 PSUM
    accumulation.  cat^T and x^T stay resident in SBUF; quantization noise
    washes out in the 2048-element softmax sums (measured ~6e-5 rel err).
  - |con|/norm4 <= 1 always (norm4 >= max|con|), so softmax needs no
    max-subtraction: e = exp(con * inv4), p = e / Z.
  - s4 = sum con^4 partials via a runtime-registered custom DVE op
    (out=(x^2)^2, accum=sum) - one Vector pass per chunk.
  - inv4 = s4^(-1/4) via exponent-shift seed + 2 Newton steps on DVE (no
    Ln/Sqrt -> a single ACT table set for the whole kernel; the ACT
    accumulator path is avoided everywhere: each read costs a ~1.5us flush).
  - e ships to DRAM; the host does the w_s/F_s/Z sums, theta, bias and
    sigmoid in f64 (the n-permutation puts y==1 first per source chunk so
    w_s is a prefix sum; DMA queues are otherwise idle).
  - PE clock warm-up: junk matmuls during the DMA fill hold the HAM gate at
    2.4 GHz so the real stream never runs throttled.
  - Engine split: PSUM drains fp32->bf16 con copies on ACT (phases A/B) and
    DVE (late phases, where ACT runs the exps); pass2 work is spliced into
    the NEXT phase's emission slots because engine queues are strict FIFO.
"""
import os

os.environ.setdefault("JAX_PLATFORMS", "axon,cpu")

from contextlib import ExitStack

import ml_dtypes
import numpy as np

import operator

import concourse.bass as bass  # noqa: F401
import concourse.tile as tile
from concourse import bacc, bass_utils, mybir
from concourse import dve_ops as _dve_ops
from concourse.dve_spec import C0 as _C0
from concourse.dve_spec import Spec as _Spec
from concourse.dve_spec import Src0 as _Src0
from concourse.dve_spec import lower as _dve_lower
from concourse.dve_spec import sq as _sq
from concourse.dve_table_gen import dve_ver_for as _dve_ver_for
from concourse.dve_uop import DveOpSpec as _DveOpSpec


def _get_quad_op():
    """Register (once, at runtime) a custom DVE op computing
    out = (in0^2)^2 with accum_out = c0 + sum(out) — the s4 partial sums in a
    single Vector pass instead of square + square-accumulate."""
    name = "QUAD_REDUCE_ANT_K"
    for o in _dve_ops.OPS:
        if o.name == name:
            return o
    spec = _Spec(
        body=_sq(_sq(_Src0)),
        accum=operator.add,
        accum_init=_C0,
        reference=_dve_ops._ref_body_sum(
            lambda in0, in1, c0, c1, c2: np.square(np.square(in0.astype(np.float32)))
        ),
    )
    row = _dve_ops._CUSTOM_DVE_ROW_BASE + len(_dve_ops.OPS)
    _dve_ops._SUB_OPCODE_FOR_NAME[name] = row
    ver = _dve_ver_for("TRN2")
    sha = _DveOpSpec(
        name=name, opcode=row, uops=_dve_lower(spec, ver=ver), rd1_en=False
    ).sha(ver)
    op = _dve_ops.DveOp(
        name, spec, subdim=False, uops_sha={ver: sha}, perf_en={ver: True}
    )
    _dve_ops.OPS.append(op)
    _dve_ops.CUSTOM_DVE_SPECS[name] = spec
    return op

B, D, N, S = 4096, 768, 8192, 4
NCORES = 8
P = 128
BL = B // NCORES          # 512 batch columns per core
NBT = BL // P             # 4 b-tiles per core
NDC = D // P              # 6 contraction chunks
CHUNK = N // S            # 2048 (source chunk along n)
G8 = 2048                 # psum drain chunk along n
NG8 = N // G8             # 4

# Magic constant for the y0 ~= x^(-1/4) exponent trick (fast-inverse-sqrt
# style): bits(y0) = K - (bits(x) >> 2).
_QROOT_K = int(round(1.25 * (2 ** 23) * (127 - 0.0450466)))

_F32 = mybir.dt.float32
_BF16 = mybir.dt.bfloat16
_I32 = mybir.dt.int32
_FP8 = mybir.dt.float8e4

_QUAD = _get_quad_op()

_cache: dict = {}


def _emit(ctx, tc, xT, catT, e_out, ks):
    nc = tc.nc
    AX = mybir.AxisListType.X
    OP = mybir.AluOpType
    AF = mybir.ActivationFunctionType

    cat_pool = ctx.enter_context(tc.tile_pool(name="cat", bufs=4))
    x_pool = ctx.enter_context(tc.tile_pool(name="xp", bufs=1))
    con_pool = ctx.enter_context(tc.tile_pool(name="conp", bufs=NBT))
    dmp_pool = ctx.enter_context(tc.tile_pool(name="dmp", bufs=3))
    e_pool = ctx.enter_context(tc.tile_pool(name="ep", bufs=2))
    st_pool = ctx.enter_context(tc.tile_pool(name="st", bufs=1))
    ps_pool = ctx.enter_context(tc.tile_pool(name="ps", bufs=2, space="PSUM"))

    # x^T resident: xT_sb[p, dc*BL + b] = xT[dc*128+p, b]
    xT_sb = x_pool.tile([P, NDC * BL], _FP8, name="xT_sb")
    for dc in range(NDC):
        nc.sync.dma_start(xT_sb[:, dc * BL:(dc + 1) * BL], xT[dc * P:(dc + 1) * P, :])

    # cat^T resident: [128, 2048] per (g4, dc).  The first g4 group is pulled
    # in 512-wide quarters so the first accumulation groups can start sooner.
    cat_sb = {}
    for g4 in range(4):
        cat_sb[g4] = cat_pool.tile([P, NDC * 2048], _FP8, name=f"cat_{g4}", tag="cat")
    for dc in range(NDC):
        for q in range(2):
            nc.sync.dma_start(
                cat_sb[0][:, dc * 2048 + q * 1024:dc * 2048 + (q + 1) * 1024],
                catT[dc * P:(dc + 1) * P, q * 1024:(q + 1) * 1024],
            )
    for g4 in range(1, 4):
        for dc in range(NDC):
            nc.sync.dma_start(
                cat_sb[g4][:, dc * 2048:(dc + 1) * 2048],
                catT[dc * P:(dc + 1) * P, g4 * 2048:(g4 + 1) * 2048],
            )

    # PE clock warm-up: the HAM gate holds a cold PE at 1.2 GHz until ~3.4us
    # of sustained activity, and re-throttles after ~3.4us idle.  Run junk
    # matmuls against the early-arriving xT tile through the cat DMA fill so
    # the real stream starts (and stays) at 2.4 GHz.
    warm_ps = ps_pool.tile([P, 512], _F32, name="warm_ps", tag="ps")
    for _ in range(18):
        nc.tensor.matmul(warm_ps, xT_sb[:, 0:P], xT_sb[:, 0:512], start=True, stop=True)
    warm_sink = st_pool.tile([P, 1], _F32, name="warm_sink")
    nc.vector.tensor_copy(warm_sink, warm_ps[:, 0:1])

    con_sb = [con_pool.tile([P, N], _BF16, name=f"con{bt}", tag="con") for bt in range(NBT)]
    s4p = [st_pool.tile([P, NG8], _F32, name=f"s4p{bt}") for bt in range(NBT)]

    def mm_chunk(bt, g8):
        """12 DoubleRow matmuls accumulating con[bt, g8*2048:(g8+1)*2048] in
        PSUM (3 k-chunks of 256 x 4 n-slices of 512), then drain: fp32->bf16
        con copy (ACT or DVE) + fused quad-reduce s4 partial (DVE)."""
        ps = ps_pool.tile([P, G8], _F32, name="ps")
        xT_r = xT_sb.rearrange("p (c b) -> p c b", c=NDC)
        cat_r = cat_sb[g8].rearrange("p (c n) -> p c n", c=NDC)
        for dc in range(NDC // 2):
            lhsT = xT_r[:, 2 * dc:2 * dc + 2, bt * P:(bt + 1) * P]
            for h in range(4):
                nc.tensor.matmul(
                    ps[:, h * 512:(h + 1) * 512],
                    lhsT,
                    cat_r[:, 2 * dc:2 * dc + 2, h * 512:(h + 1) * 512],
                    start=(dc == 0),
                    stop=(dc == NDC // 2 - 1),
                    perf_mode=mybir.MatmulPerfMode.DoubleRow,
                )
        cs = con_sb[bt][:, g8 * G8:(g8 + 1) * G8]
        # Phase B/C drains go to DVE: ACT is busy with the softmax exps there,
        # while DVE only runs the fused quad.
        if bt >= 2 and g8 >= 2:
            nc.vector.tensor_copy(cs, ps)
        else:
            nc.scalar.activation(cs, ps, AF.Copy)
        # s4 partial in ONE fused DVE pass (bf16 con input -> 2x perf mode).
        dmp = dmp_pool.tile([P, G8], _BF16, name="dmp")
        nc.vector._custom_dve(
            _QUAD, out=dmp, in0=cs, s0=0.0, s1=0.0, imm2=0.0,
            accum_out=s4p[bt][:, g8:g8 + 1],
        )

    def newton_closure(bt):
        def emit():
            # s4 = sum of chunk partials; inv4 = s4^(-1/4) via bit trick + Newton.
            eng = nc.vector
            s4 = st_pool.tile([P, 1], _F32, name=f"s4_{bt}")
            nc.vector.tensor_reduce(s4, s4p[bt], axis=AX, op=OP.add)
            eng.tensor_scalar_max(s4, s4, 1e-30)
            y = st_pool.tile([P, 1], _F32, name=f"y_{bt}")
            eng.tensor_scalar(y.bitcast(_I32), s4.bitcast(_I32), 2, None,
                              op0=OP.arith_shift_right)
            eng.tensor_scalar(y.bitcast(_I32), y.bitcast(_I32), -1, _QROOT_K,
                              op0=OP.mult, op1=OP.add)
            y2 = st_pool.tile([P, 1], _F32, name=f"y2_{bt}")
            u = st_pool.tile([P, 1], _F32, name=f"u_{bt}")
            for _ in range(2):
                eng.tensor_tensor(y2, y, y, op=OP.mult)
                eng.tensor_tensor(y2, y2, y2, op=OP.mult)      # y^4
                eng.tensor_tensor(u, y2, s4, op=OP.mult)       # a*y^4
                eng.tensor_scalar(u, u, -0.25, 1.25, op0=OP.mult, op1=OP.add)
                eng.tensor_tensor(y, y, u, op=OP.mult)
            inv4[bt] = y
        return emit

    def exp_closure(bt):
        def emit():
            # One exp over the whole b-tile row (plain ACTIVATE -- the
            # accumulator path costs a ~1.5us pipeline flush per read); e
            # ships to DRAM in quarter DMAs and the host does the w/F sums
            # in f64 (DMA queues are otherwise idle).
            y = inv4[bt]
            e = e_pool.tile([P, N], _BF16, name="e", tag="e")
            nc.scalar.activation(e, con_sb[bt], AF.Exp, scale=y)
            for q in range(4):
                nc.sync.dma_start(
                    e_out[:, bt * N + q * CHUNK:bt * N + (q + 1) * CHUNK],
                    e[:, q * CHUNK:(q + 1) * CHUNK],
                )
        return emit

    inv4 = {}
    # Emission order == engine FIFO order (Tile schedules by program order).
    # pass2 work of a phase is spliced into the NEXT phase's chunk slots so
    # exp ops never sit in ACT's FIFO ahead of psum-drain copies whose psum
    # slots the TensorEngine is waiting to reuse.
    pending = []
    for phase in ((0, 1), (2,), (3,)):
        for g8 in range(NG8):
            for bt in phase:
                mm_chunk(bt, g8)
            if pending:
                pending.pop(0)()
        for bt in phase:
            pending.append(newton_closure(bt))
            pending.append(exp_closure(bt))
    for fn in pending:
        fn()



def build_program(ks=None):
    key = "prog"
    if key in _cache:
        return _cache[key]
    nc = bacc.Bacc("TRN2", target_bir_lowering=False, debug=False, num_devices=NCORES)
    xT = nc.dram_tensor("xTl", [D, BL], _FP8, kind="ExternalInput").ap()
    catT = nc.dram_tensor("catTp", [D, N], _FP8, kind="ExternalInput").ap()
    e_out = nc.dram_tensor("e_out", [P, NBT * N], _BF16, kind="ExternalOutput").ap()
    with tile.TileContext(nc) as tc, ExitStack() as ctx:
        _emit(ctx, tc, xT, catT, e_out, ks)
    nc.compile()
    _cache[key] = nc
    return nc


def host_prep(batch_x, cat, y):
    """Permute n within each source chunk (y==1 first), build bf16 transposed
    inputs. Returns (catT_bf16 [D,N], xT_bf16 [D,B], ks)."""
    y = np.asarray(y)
    perm = np.empty(N, dtype=np.int64)
    ks = []
    for s in range(S):
        ys = y[s * CHUNK:(s + 1) * CHUNK]
        order = np.argsort(ys == 0, kind="stable")  # nonzero first
        perm[s * CHUNK:(s + 1) * CHUNK] = s * CHUNK + order
        ks.append(int((ys != 0).sum()))
    catp = np.asarray(cat)[perm]
    catT = np.ascontiguousarray(catp.T).astype(ml_dtypes.float8_e4m3)
    xT = np.ascontiguousarray(np.asarray(batch_x).T).astype(ml_dtypes.float8_e4m3)
    return catT, xT, ks


def host_epilogue(results, batch_x, phi, bias, ks):
    """results: list over cores of {'e_out': [128, NBT*N] bf16}. Host computes
    w_s (prefix sums), F_s, Z, theta, bias, sigmoid in f64."""
    theta = np.exp(np.asarray(batch_x, np.float64) @ np.asarray(phi, np.float64).T)
    out = np.empty(B, np.float64)
    for c in range(NCORES):
        e = np.asarray(results[c]["e_out"]).astype(np.float64)
        e = e.reshape(P, NBT, S, CHUNK)
        f = e.sum(axis=3)                       # [P, NBT, S]
        z = f.sum(axis=2)                       # [P, NBT]
        for bt in range(NBT):
            bidx = c * BL + bt * P + np.arange(P)
            w = np.stack(
                [e[:, bt, s, :ks[s]].sum(axis=1) for s in range(S)], axis=1
            )                                   # [P, S]
            out[bidx] = ((w / z[:, bt:bt + 1]) * theta[bidx, :]).sum(axis=1)
    out = out + float(np.asarray(bias).reshape(-1)[0])
    return (1.0 / (1.0 + np.exp(-out))).astype(np.float32)


def make_in_maps(catT, xT):
    return [
        {
            "catTp": catT,
            "xTl": np.ascontiguousarray(xT[:, c * BL:(c + 1) * BL]),
        }
        for c in range(NCORES)
    ]


def kernel(batch_x, cat, y, phi, bias):
    catT, xT, ks = host_prep(batch_x, cat, y)
    nc = build_program(ks)
    res = bass_utils.run_bass_kernel_spmd(nc, make_in_maps(catT, xT), core_ids=list(range(NCORES)))
    return host_epilogue(res.results, batch_x, phi, bias, ks)


# revision 34
# speedup vs baseline: 1.1060x; 1.0966x over previous
"""Trainium2 Bass kernel for nn_DomainAttention (moe_routing).

Math (see reference):
    con[n,b]  = cat[n] . x[b]                       # [N, B]
    con      /= max(||con[:,b]||_4, 1e-12)          # 4-norm over N, per column
    p         = softmax(con, axis=N)
    w[s,b]    = sum_{n in chunk s} y[n] * p[n,b]
    theta[s,b]= exp(x[b] . phi[s])
    out[b]    = sigmoid(sum_s w[s,b]*theta[s,b] + bias)

Device strategy (8 NeuronCores, data-parallel over B, 512 columns/core):
  - con computed as [b_part=128, n_free] tiles: lhsT = x^T (stationary), rhs =
    cat^T (moving), fp8e4m3 inputs with DoubleRow perf mode (two 128-deep
    contraction sub-rows per matmul -> half the matmul count), fp32 PSUM
    accumulation.  cat^T and x^T stay resident in SBUF; quantization noise
    washes out in the 2048-element softmax sums (measured ~6e-5 rel err).
  - |con|/norm4 <= 1 always (norm4 >= max|con|), so softmax needs no
    max-subtraction: e = exp(con * inv4), p = e / Z.
  - s4 = sum con^4 partials via a runtime-registered custom DVE op
    (out=(x^2)^2, accum=sum) - one Vector pass per chunk.
  - inv4 = s4^(-1/4) via exponent-shift seed + 2 Newton steps on DVE (no
    Ln/Sqrt -> a single ACT table set for the whole kernel; the ACT
    accumulator path is avoided everywhere: each read costs a ~1.5us flush).
  - e ships to DRAM; the host does the w_s/F_s/Z sums, theta, bias and
    sigmoid in f64 (the n-permutation puts y==1 first per source chunk so
    w_s is a prefix sum; DMA queues are otherwise idle).
  - PE clock warm-up: junk matmuls during the DMA fill hold the HAM gate at
    2.4 GHz so the real stream never runs throttled.
  - Engine split: PSUM drains fp32->bf16 con copies on ACT (phases A/B) and
    DVE (late phases, where ACT runs the exps); pass2 work is spliced into
    the NEXT phase's emission slots because engine queues are strict FIFO.
"""
import os

os.environ.setdefault("JAX_PLATFORMS", "axon,cpu")

from contextlib import ExitStack

import ml_dtypes
import numpy as np

import operator

import concourse.bass as bass  # noqa: F401
import concourse.tile as tile
from concourse import bacc, bass_utils, mybir
from concourse import dve_ops as _dve_ops
from concourse.dve_spec import C0 as _C0
from concourse.dve_spec import Spec as _Spec
from concourse.dve_spec import Src0 as _Src0
from concourse.dve_spec import lower as _dve_lower
from concourse.dve_spec import sq as _sq
from concourse.dve_table_gen import dve_ver_for as _dve_ver_for
from concourse.dve_uop import DveOpSpec as _DveOpSpec


def _get_quad_op():
    """Register (once, at runtime) a custom DVE op computing
    out = (in0^2)^2 with accum_out = c0 + sum(out) — the s4 partial sums in a
    single Vector pass instead of square + square-accumulate."""
    name = "QUAD_REDUCE_ANT_K"
    for o in _dve_ops.OPS:
        if o.name == name:
            return o
    spec = _Spec(
        body=_sq(_sq(_Src0)),
        accum=operator.add,
        accum_init=_C0,
        reference=_dve_ops._ref_body_sum(
            lambda in0, in1, c0, c1, c2: np.square(np.square(in0.astype(np.float32)))
        ),
    )
    row = _dve_ops._CUSTOM_DVE_ROW_BASE + len(_dve_ops.OPS)
    _dve_ops._SUB_OPCODE_FOR_NAME[name] = row
    ver = _dve_ver_for("TRN2")
    sha = _DveOpSpec(
        name=name, opcode=row, uops=_dve_lower(spec, ver=ver), rd1_en=False
    ).sha(ver)
    op = _dve_ops.DveOp(
        name, spec, subdim=False, uops_sha={ver: sha}, perf_en={ver: True}
    )
    _dve_ops.OPS.append(op)
    _dve_ops.CUSTOM_DVE_SPECS[name] = spec
    return op

B, D, N, S = 4096, 768, 8192, 4
NCORES = 8
P = 128
BL = B // NCORES          # 512 batch columns per core
NBT = BL // P             # 4 b-tiles per core
NDC = D // P              # 6 contraction chunks
CHUNK = N // S            # 2048 (source chunk along n)
G8 = 2048                 # psum drain chunk along n
NG8 = N // G8             # 4

# Magic constant for the y0 ~= x^(-1/4) exponent trick (fast-inverse-sqrt
# style): bits(y0) = K - (bits(x) >> 2).
_QROOT_K = int(round(1.25 * (2 ** 23) * (127 - 0.0450466)))

_F32 = mybir.dt.float32
_BF16 = mybir.dt.bfloat16
_I32 = mybir.dt.int32
_FP8 = mybir.dt.float8e4

_QUAD = _get_quad_op()

_cache: dict = {}


def _emit(ctx, tc, xT, catT, e_out, ks):
    nc = tc.nc
    AX = mybir.AxisListType.X
    OP = mybir.AluOpType
    AF = mybir.ActivationFunctionType

    cat_pool = ctx.enter_context(tc.tile_pool(name="cat", bufs=4))
    x_pool = ctx.enter_context(tc.tile_pool(name="xp", bufs=1))
    con_pool = ctx.enter_context(tc.tile_pool(name="conp", bufs=NBT))
    dmp_pool = ctx.enter_context(tc.tile_pool(name="dmp", bufs=3))
    e_pool = ctx.enter_context(tc.tile_pool(name="ep", bufs=2))
    st_pool = ctx.enter_context(tc.tile_pool(name="st", bufs=1))
    ps_pool = ctx.enter_context(tc.tile_pool(name="ps", bufs=2, space="PSUM"))

    # x^T resident: xT_sb[p, dc*BL + b] = xT[dc*128+p, b]
    xT_sb = x_pool.tile([P, NDC * BL], _FP8, name="xT_sb")
    for dc in range(NDC):
        nc.sync.dma_start(xT_sb[:, dc * BL:(dc + 1) * BL], xT[dc * P:(dc + 1) * P, :])

    # cat^T resident: [128, 2048] per (g4, dc).  The first g4 group is pulled
    # in 512-wide quarters so the first accumulation groups can start sooner.
    cat_sb = {}
    for g4 in range(4):
        cat_sb[g4] = cat_pool.tile([P, NDC * 2048], _FP8, name=f"cat_{g4}", tag="cat")
    for dc in range(NDC):
        for q in range(2):
            nc.sync.dma_start(
                cat_sb[0][:, dc * 2048 + q * 1024:dc * 2048 + (q + 1) * 1024],
                catT[dc * P:(dc + 1) * P, q * 1024:(q + 1) * 1024],
            )
    for g4 in range(1, 4):
        for dc in range(NDC):
            nc.sync.dma_start(
                cat_sb[g4][:, dc * 2048:(dc + 1) * 2048],
                catT[dc * P:(dc + 1) * P, g4 * 2048:(g4 + 1) * 2048],
            )

    # PE clock warm-up: the HAM gate holds a cold PE at 1.2 GHz until ~3.4us
    # of sustained activity, and re-throttles after ~3.4us idle.  Run junk
    # matmuls against the early-arriving xT tile through the cat DMA fill so
    # the real stream starts (and stays) at 2.4 GHz.
    warm_ps = ps_pool.tile([P, 512], _F32, name="warm_ps", tag="ps")
    for _ in range(18):
        nc.tensor.matmul(warm_ps, xT_sb[:, 0:P], xT_sb[:, 0:512], start=True, stop=True)
    warm_sink = st_pool.tile([P, 1], _F32, name="warm_sink")
    nc.vector.tensor_copy(warm_sink, warm_ps[:, 0:1])

    con_sb = [con_pool.tile([P, N], _BF16, name=f"con{bt}", tag="con") for bt in range(NBT)]
    s4p = [st_pool.tile([P, NG8], _F32, name=f"s4p{bt}") for bt in range(NBT)]

    def mm_chunk(bt, g8):
        """12 DoubleRow matmuls accumulating con[bt, g8*2048:(g8+1)*2048] in
        PSUM (3 k-chunks of 256 x 4 n-slices of 512), then drain: fp32->bf16
        con copy (ACT or DVE) + fused quad-reduce s4 partial (DVE)."""
        ps = ps_pool.tile([P, G8], _F32, name="ps")
        xT_r = xT_sb.rearrange("p (c b) -> p c b", c=NDC)
        cat_r = cat_sb[g8].rearrange("p (c n) -> p c n", c=NDC)
        for dc in range(NDC // 2):
            lhsT = xT_r[:, 2 * dc:2 * dc + 2, bt * P:(bt + 1) * P]
            for h in range(4):
                nc.tensor.matmul(
                    ps[:, h * 512:(h + 1) * 512],
                    lhsT,
                    cat_r[:, 2 * dc:2 * dc + 2, h * 512:(h + 1) * 512],
                    start=(dc == 0),
                    stop=(dc == NDC // 2 - 1),
                    perf_mode=mybir.MatmulPerfMode.DoubleRow,
                )
        cs = con_sb[bt][:, g8 * G8:(g8 + 1) * G8]
        # Phase B/C drains go to DVE: ACT is busy with the softmax exps there,
        # while DVE only runs the fused quad.
        nc.scalar.activation(cs, ps, AF.Copy)
        # s4 partial in ONE fused DVE pass (bf16 con input -> 2x perf mode).
        dmp = dmp_pool.tile([P, G8], _BF16, name="dmp")
        nc.vector._custom_dve(
            _QUAD, out=dmp, in0=cs, s0=0.0, s1=0.0, imm2=0.0,
            accum_out=s4p[bt][:, g8:g8 + 1],
        )

    def newton_closure(bt):
        def emit():
            # s4 = sum of chunk partials; inv4 = s4^(-1/4) via bit trick + Newton.
            eng = nc.vector
            s4 = st_pool.tile([P, 1], _F32, name=f"s4_{bt}")
            nc.vector.tensor_reduce(s4, s4p[bt], axis=AX, op=OP.add)
            eng.tensor_scalar_max(s4, s4, 1e-30)
            y = st_pool.tile([P, 1], _F32, name=f"y_{bt}")
            eng.tensor_scalar(y.bitcast(_I32), s4.bitcast(_I32), 2, None,
                              op0=OP.arith_shift_right)
            eng.tensor_scalar(y.bitcast(_I32), y.bitcast(_I32), -1, _QROOT_K,
                              op0=OP.mult, op1=OP.add)
            y2 = st_pool.tile([P, 1], _F32, name=f"y2_{bt}")
            u = st_pool.tile([P, 1], _F32, name=f"u_{bt}")
            for _ in range(2):
                eng.tensor_tensor(y2, y, y, op=OP.mult)
                # u = (y2 * s4) * y2 = s4*y^4 in one op (s4 rides the
                # per-partition scalar port)
                eng.scalar_tensor_tensor(out=u, in0=y2, scalar=s4, in1=y2,
                                         op0=OP.mult, op1=OP.mult)
                eng.tensor_scalar(u, u, -0.25, 1.25, op0=OP.mult, op1=OP.add)
                eng.tensor_tensor(y, y, u, op=OP.mult)
            inv4[bt] = y
        return emit

    def exp_closure(bt):
        def emit():
            # One exp over the whole b-tile row (plain ACTIVATE -- the
            # accumulator path costs a ~1.5us pipeline flush per read); e
            # ships to DRAM in quarter DMAs and the host does the w/F sums
            # in f64 (DMA queues are otherwise idle).
            y = inv4[bt]
            e = e_pool.tile([P, N], _BF16, name="e", tag="e")
            nc.scalar.activation(e, con_sb[bt], AF.Exp, scale=y)
            for q in range(4):
                nc.sync.dma_start(
                    e_out[:, bt * N + q * CHUNK:bt * N + (q + 1) * CHUNK],
                    e[:, q * CHUNK:(q + 1) * CHUNK],
                )
        return emit

    inv4 = {}
    # Emission order == engine FIFO order (Tile schedules by program order).
    # pass2 work of a phase is spliced into the NEXT phase's chunk slots so
    # exp ops never sit in ACT's FIFO ahead of psum-drain copies whose psum
    # slots the TensorEngine is waiting to reuse.
    pending = []
    for phase in ((0, 1), (2,), (3,)):
        for g8 in range(NG8):
            for bt in phase:
                mm_chunk(bt, g8)
            if pending:
                pending.pop(0)()
        for bt in phase:
            pending.append(newton_closure(bt))
            pending.append(exp_closure(bt))
    for fn in pending:
        fn()



def build_program(ks=None):
    key = "prog"
    if key in _cache:
        return _cache[key]
    nc = bacc.Bacc("TRN2", target_bir_lowering=False, debug=False, num_devices=NCORES)
    xT = nc.dram_tensor("xTl", [D, BL], _FP8, kind="ExternalInput").ap()
    catT = nc.dram_tensor("catTp", [D, N], _FP8, kind="ExternalInput").ap()
    e_out = nc.dram_tensor("e_out", [P, NBT * N], _BF16, kind="ExternalOutput").ap()
    with tile.TileContext(nc) as tc, ExitStack() as ctx:
        _emit(ctx, tc, xT, catT, e_out, ks)
    nc.compile()
    _cache[key] = nc
    return nc


def host_prep(batch_x, cat, y):
    """Permute n within each source chunk (y==1 first), build bf16 transposed
    inputs. Returns (catT_bf16 [D,N], xT_bf16 [D,B], ks)."""
    y = np.asarray(y)
    perm = np.empty(N, dtype=np.int64)
    ks = []
    for s in range(S):
        ys = y[s * CHUNK:(s + 1) * CHUNK]
        order = np.argsort(ys == 0, kind="stable")  # nonzero first
        perm[s * CHUNK:(s + 1) * CHUNK] = s * CHUNK + order
        ks.append(int((ys != 0).sum()))
    catp = np.asarray(cat)[perm]
    catT = np.ascontiguousarray(catp.T).astype(ml_dtypes.float8_e4m3)
    xT = np.ascontiguousarray(np.asarray(batch_x).T).astype(ml_dtypes.float8_e4m3)
    return catT, xT, ks


def host_epilogue(results, batch_x, phi, bias, ks):
    """results: list over cores of {'e_out': [128, NBT*N] bf16}. Host computes
    w_s (prefix sums), F_s, Z, theta, bias, sigmoid in f64."""
    theta = np.exp(np.asarray(batch_x, np.float64) @ np.asarray(phi, np.float64).T)
    out = np.empty(B, np.float64)
    for c in range(NCORES):
        e = np.asarray(results[c]["e_out"]).astype(np.float64)
        e = e.reshape(P, NBT, S, CHUNK)
        f = e.sum(axis=3)                       # [P, NBT, S]
        z = f.sum(axis=2)                       # [P, NBT]
        for bt in range(NBT):
            bidx = c * BL + bt * P + np.arange(P)
            w = np.stack(
                [e[:, bt, s, :ks[s]].sum(axis=1) for s in range(S)], axis=1
            )                                   # [P, S]
            out[bidx] = ((w / z[:, bt:bt + 1]) * theta[bidx, :]).sum(axis=1)
    out = out + float(np.asarray(bias).reshape(-1)[0])
    return (1.0 / (1.0 + np.exp(-out))).astype(np.float32)


def make_in_maps(catT, xT):
    return [
        {
            "catTp": catT,
            "xTl": np.ascontiguousarray(xT[:, c * BL:(c + 1) * BL]),
        }
        for c in range(NCORES)
    ]


def kernel(batch_x, cat, y, phi, bias):
    catT, xT, ks = host_prep(batch_x, cat, y)
    nc = build_program(ks)
    res = bass_utils.run_bass_kernel_spmd(nc, make_in_maps(catT, xT), core_ids=list(range(NCORES)))
    return host_epilogue(res.results, batch_x, phi, bias, ks)
